# revision 1
# baseline (speedup 1.0000x reference)
"""nn_AttentionAverageStdScalingModule kernel.

Self-contained: takes FULL unsharded inputs, returns FULL output.
Shapes are hardcoded per the problem spec:
  test_scores  (1, 16, 88, 88)   f32
  train_labels (30, 16, 88, 88)  f32
  test_feat    (1, 16, 256, 22, 22)  f32
  train_feats  (30, 16, 256, 22, 22) f32
  softmax_temp (1,) f32

The nseq axis (16) is the independent axis; work is processed per-sequence
(the 8-way nseq sharding used for the device path maps 2 sequences per core).
This implementation computes the full pipeline in fp32:
  cosine similarity -> temperature softmax over each memory's pixels ->
  label aggregation -> bilinear 22->88 upsample -> cross-memory mean/std ->
  exp certainty rescale + residual.
"""

import numpy as np

NMEM, NSEQ, C, WF, HF = 30, 16, 256, 22, 22
WL, HL = 88, 88
P2 = WF * HF
ALPHA = 20.0


def _resize_matrix(n_in: int, n_out: int) -> np.ndarray:
    """Row-stochastic 1-D bilinear resize matrix matching
    jax.image.resize(method='bilinear', antialias=False) semantics
    (half-pixel centers, out-of-range taps dropped and renormalized)."""
    M = np.zeros((n_out, n_in), np.float64)
    scale = n_in / n_out
    for i in range(n_out):
        x = (i + 0.5) * scale - 0.5
        x0 = int(np.floor(x))
        for tap, w in ((x0, 1.0 - (x - x0)), (x0 + 1, x - x0)):
            if 0 <= tap < n_in and w > 0.0:
                M[i, tap] += w
        s = M[i].sum()
        if s > 0:
            M[i] /= s
    return M.astype(np.float32)


_DN = _resize_matrix(WL, WF)   # (22, 88) downsample
_UP = _resize_matrix(WF, WL)   # (88, 22) upsample


def kernel(test_scores, train_labels, test_feat, train_feats, softmax_temp):
    test_scores = np.asarray(test_scores, np.float32)
    train_labels = np.asarray(train_labels, np.float32)
    test_feat = np.asarray(test_feat, np.float32)
    train_feats = np.asarray(train_feats, np.float32)
    temp = np.float32(np.asarray(softmax_temp).reshape(-1)[0])

    # Downsample labels to feature resolution: (30, 16, 22, 22) -> flat (30,16,484)
    labels_down = np.einsum(
        'ij,mnjk,lk->mnil', _DN, train_labels, _DN, optimize=True
    ).reshape(NMEM, NSEQ, P2)

    out = np.empty((1, NSEQ, WL, HL), np.float32)

    for s in range(NSEQ):
        te = test_feat[0, s].reshape(C, P2)                    # (256, 484)
        tr = train_feats[:, s].transpose(1, 0, 2, 3).reshape(C, NMEM * P2)

        te_n = te / np.sqrt((te * te).sum(axis=0, keepdims=True))
        tr_n = tr / np.sqrt((tr * tr).sum(axis=0, keepdims=True))

        # cosine similarity (484 test pixels x 30*484 memory pixels)
        sim = te_n.T @ tr_n                                     # (484, 30*484)
        sim = sim.reshape(P2, NMEM, P2).transpose(1, 0, 2)      # (30, 484, 484)

        z = temp * sim
        z -= z.max(axis=2, keepdims=True)
        ez = np.exp(z, dtype=np.float32)
        p = ez / ez.sum(axis=2, keepdims=True)                  # (30, 484, 484)

        pmt_down = np.einsum('mjk,mk->mj', p, labels_down[:, s], optimize=True)
        pmt_down = pmt_down.reshape(NMEM, WF, HF)

        # bilinear upsample 22x22 -> 88x88 for each memory
        pmt = np.einsum('ij,mjk,lk->mil', _UP, pmt_down, _UP, optimize=True)

        mean = pmt.mean(axis=0)
        std = pmt.std(axis=0, ddof=1)
        certainty = np.exp(ALPHA / (1.0 + std * std) - ALPHA)
        out[0, s] = certainty * mean + test_scores[0, s]

    return out



# revision 29
# speedup vs baseline: 41.4547x; 41.4547x over previous
"""nn_AttentionAverageStdScalingModule — Trainium2 Bass kernel.

Self-contained: takes FULL unsharded inputs, returns FULL output.
Shards nseq=16 across 8 NeuronCores (2 sequences per core); each core runs an
identical Bass program on its own slice.

Per (core, seq) pipeline (all on device):
  labels 88x88 --DN--> 22x22 (separable PE matmuls)         -> v (W = [v, 1])
  tr (256, 30*484) col norms: DVE squares -> PE matmul (squares stationary,
      ones moving) -> j-on-partition norms -> ACT ln/exp -> r_j = 1/||tr_j||
  te (256, 484) col norms -> te' = te * softmax_temp/||te_i|| (K=1 bcast mm)
  S^T = tr^T @ te'   (float32r matmuls, j on PSUM partitions)
  E = exp(S^T * r_j) (ACT, per-partition scale; no max-subtraction needed:
      |z| <= temp so exp stays in fp32 range)
  (numer, denom)^T = E^T-stationary matmuls with [v, 1] moving (N=2)
  pmt_down^T = numer * exp(-ln denom)
  bilinear 22->88 (separable PE matmuls), cross-memory mean/var (DVE),
  certainty = exp(a/(1+var) - a) (ACT ln/exp chain), out = c*mean + ts.
"""

import numpy as np

import concourse.bass as bass
import concourse.tile as tile
from concourse import mybir
from concourse.bass_utils import run_bass_kernel_spmd

F32 = mybir.dt.float32
F32R = mybir.dt.float32r
BF16 = mybir.dt.bfloat16
AF = mybir.ActivationFunctionType

NMEM, NSEQ, C, WF, HF = 30, 16, 256, 22, 22
WL, HL = 88, 88
P2 = WF * HF            # 484
PL2 = WL * HL           # 7744
ALPHA = 20.0
NCORES = 8
SPC = NSEQ // NCORES    # 2 sequences per core
KT = C // 128           # 2 contraction tiles
# j-tiles within one memory's 484 pixels: 128,128,128,100
JTS = [(0, 128), (128, 128), (256, 128), (384, 100)]
GM = 10                 # memories per tr DMA group
NG = NMEM // GM


def _resize_matrix(n_in: int, n_out: int) -> np.ndarray:
    """Row-stochastic 1-D bilinear resize matrix matching
    jax.image.resize(method='bilinear', antialias=False)."""
    M = np.zeros((n_out, n_in), np.float64)
    scale = n_in / n_out
    for i in range(n_out):
        x = (i + 0.5) * scale - 0.5
        x0 = int(np.floor(x))
        for tap, w in ((x0, 1.0 - (x - x0)), (x0 + 1, x - x0)):
            if 0 <= tap < n_in and w > 0.0:
                M[i, tap] += w
        s = M[i].sum()
        if s > 0:
            M[i] /= s
    return M.astype(np.float32)


_DN = _resize_matrix(WL, WF)   # (22, 88)
_UP = _resize_matrix(WF, WL)   # (88, 22)


def _split_excess_waits(nc, max_waits=1):
    """Walrus encodes only one semaphore wait per instruction; Tile's
    sem-assigner can emit more. Spill excess waits onto same-engine NoOps
    inserted immediately before the offender — engines execute their stream
    in order, so the waits still happen-before the instruction."""
    n = 0
    for fn in nc.m.functions:
        for blk in fn.blocks:
            changed = False
            out = []
            for inst in blk.instructions:
                si = inst.sync_info
                if si is not None and len(si.on_wait) > max_waits:
                    waits = list(si.on_wait)
                    excess, keep = waits[:-max_waits], waits[-max_waits:]
                    for c in range(0, len(excess), max_waits):
                        nop = mybir.InstNoOp(name=f"I-waitsplit-{n}",
                                             ins=[], outs=[])
                        nop.engine = inst.engine
                        nop.sync_info = mybir.SyncInfo(
                            on_wait=excess[c:c + max_waits], on_update=[])
                        out.append(nop)
                        n += 1
                    inst.sync_info = mybir.SyncInfo(
                        on_wait=keep, on_update=list(si.on_update))
                    changed = True
                out.append(inst)
            if changed:
                blk.instructions = out
    return n


def build_program():
    nc = bass.Bass("TRN2", target_bir_lowering=False, debug=False,
                   enable_asserts=False)

    tf = nc.dram_tensor("tf", [SPC, NMEM, C, P2], F32, kind="ExternalInput")
    te = nc.dram_tensor("te", [SPC, C, P2], F32, kind="ExternalInput")
    lx = nc.dram_tensor("lx", [SPC, WL, NMEM, HL], F32, kind="ExternalInput")
    ts = nc.dram_tensor("ts", [SPC, WL, HL], F32, kind="ExternalInput")
    tmp_t = nc.dram_tensor("temp", [1, 1], F32, kind="ExternalInput")
    dnt = nc.dram_tensor("dnt", [WL, WF], F32, kind="ExternalInput")
    upt = nc.dram_tensor("upt", [WF, WL], F32, kind="ExternalInput")
    out = nc.dram_tensor("out", [SPC, WL, HL], F32, kind="ExternalOutput")
    # DRAM scratch for partition<->free refactor roundtrips, (j, m) layout
    # with j padded to 512 (rows 484.. are never read back meaningfully)
    vd = nc.dram_tensor("vd", [SPC, 512, NMEM], BF16, kind="Internal")
    pd = nc.dram_tensor("pd", [SPC, 512, NMEM], BF16, kind="Internal")

    with tile.TileContext(nc) as tc:
        with (
            tc.tile_pool(name="const", bufs=1) as const_p,
            tc.tile_pool(name="tr", bufs=2) as tr_p,
            tc.tile_pool(name="sq", bufs=1) as sq_p,
            tc.tile_pool(name="tes", bufs=2) as te_p,
            tc.tile_pool(name="lxp", bufs=1) as lx_p,
            tc.tile_pool(name="small", bufs=2) as sm_p,
            tc.tile_pool(name="ep", bufs=2) as ep_p,
            tc.tile_pool(name="epp", bufs=4) as epp_p,
            tc.tile_pool(name="ee", bufs=2) as e_p,
            tc.tile_pool(name="ps_mm1", bufs=3, space="PSUM") as ps_mm1,
            tc.tile_pool(name="ps_pd", bufs=1, space="PSUM") as ps_pd,
            tc.tile_pool(name="ps_r2", bufs=1, space="PSUM") as ps_r2,
            tc.tile_pool(name="ps_s", bufs=1, space="PSUM") as ps_s,
        ):
            # ---- constants ----
            dnt_sb = const_p.tile([WL, WF], F32R)       # DN^T (88, 22)
            nc.gpsimd.dma_start(out=dnt_sb, in_=dnt[:, :])
            dnt_bf = const_p.tile([WL, WF], BF16)
            nc.vector.tensor_copy(dnt_bf, dnt_sb)
            upt_sb = const_p.tile([WF, WL], F32)        # UP^T (22, 88)
            nc.sync.dma_start(out=upt_sb, in_=upt[:, :])
            upt_bf = const_p.tile([WF, WL], BF16)
            nc.vector.tensor_copy(upt_bf, upt_sb)
            ones_bf = const_p.tile([128, 1], BF16)
            nc.vector.memset(ones_bf, 1.0)
            ones_row = const_p.tile([1, 128], BF16)
            nc.vector.memset(ones_row, 1.0)
            nalpha = const_p.tile([WL, 1], F32)
            nc.vector.memset(nalpha, -ALPHA)
            temp_sb = const_p.tile([1, 1], F32)
            nc.sync.dma_start(out=temp_sb, in_=tmp_t[:, :])
            lnt = const_p.tile([1, 1], F32)             # ln(softmax_temp)
            nc.scalar.activation(lnt, temp_sb, AF.Ln)

            for s in range(SPC):
                # ============ test-feature side ============
                te_sb = te_p.tile([128, KT, P2], F32, tag="te_sb")
                nc.sync.dma_start(
                    out=te_sb,
                    in_=te[s].rearrange("(kt p) j -> p kt j", p=128))
                tesq = te_p.tile([128, KT, P2], BF16, tag="tesq")
                nc.vector.tensor_mul(tesq, te_sb, te_sb)
                # ||te_i||^2 -> (1, 484) psum
                nte = ps_s.tile([1, P2], F32, tag="scr")
                for kt in range(KT):
                    nc.tensor.matmul(nte, ones_bf, tesq[:, kt, :],
                                     start=(kt == 0), stop=(kt == KT - 1))
                lnte = sm_p.tile([1, P2], F32, tag="lnte")
                nc.scalar.activation(lnte, nte, AF.Ln)
                # temp / ||te_i||  =  exp(-0.5*ln(n^2) + ln(temp))
                rte = sm_p.tile([1, P2], BF16, tag="rte")
                nc.scalar.activation(rte, lnte, AF.Exp, bias=lnt[:1, :],
                                     scale=-0.5)
                # broadcast to 128 partitions via K=1 matmul
                rteb = ps_s.tile([128, P2], F32, tag="scr")
                nc.tensor.matmul(rteb, ones_row[:1, :], rte[:1, :],
                                 start=True, stop=True)
                te2 = te_p.tile([128, KT, P2], F32R, tag="te2")
                for kt in range(KT):
                    nc.vector.tensor_mul(te2[:, kt, :], te_sb[:, kt, :], rteb)

                # ============ labels downsample ============
                lx_sb = lx_p.tile([WL, NMEM, HL], F32R)
                nc.gpsimd.dma_start(out=lx_sb, in_=lx[s])
                # stage 1: per-m stationary (xL,yL), contract xL:
                #   D1'[m] = lx[:, m, :]^T @ DN^T  -> (yL=88, x=22)
                d1sb = sm_p.tile([WL, NMEM, WF], BF16, tag="d1sb")
                for half in range(2):
                    d1ps = ps_s.tile([WL, 15, 32], F32, tag="scr")
                    for mi in range(15):
                        m = half * 15 + mi
                        nc.tensor.matmul(
                            d1ps[:, mi, :WF],
                            lx_sb[:, m, :],
                            dnt_sb,
                            start=True, stop=True)
                    nc.vector.tensor_copy(d1sb[:, half * 15:(half + 1) * 15, :],
                                          d1ps[:, :, :WF])
                # stage 2: contract yL: v = DN @ D1' -> (y=22, (x,m))
                v_sb = sm_p.tile([WF, WF, NMEM], BF16, tag="v_sb")
                for cch in range(2):
                    vps = ps_s.tile([WF, 2, 512], F32, tag="scr")
                    nc.tensor.matmul(
                        vps[:, cch, :330],
                        dnt_bf,
                        d1sb[:, cch * 15:(cch + 1) * 15, :].rearrange(
                            "p m x -> p x m"),
                        start=True, stop=True)
                    nc.vector.tensor_copy(
                        v_sb[:, :, cch * 15:(cch + 1) * 15],
                        vps[:, cch, :330].rearrange("p (x m) -> p x m", x=WF))
                # roundtrip through DRAM to reorder (x*22+y) onto partitions
                nc.sync.dma_start(
                    out=vd[s, 0:P2].rearrange("(x y) m -> y x m", y=WF),
                    in_=v_sb)
                w_sb = sm_p.tile([128, 4, 2, NMEM], BF16, tag="w_sb")
                nc.vector.memset(w_sb[:, :, 1, :], 1.0)
                nc.sync.dma_start(
                    out=w_sb[:, :, 0, :],
                    in_=vd[s].rearrange("(jt p) m -> p jt m", p=128))

                # ============ memory loop ============
                r2 = ps_r2.tile([128, 4, NMEM], F32)
                pdt = ps_pd.tile([128, 4, NMEM, 2], F32)
                r_sb = sm_p.tile([128, 4, NMEM], F32, tag="r_sb")
                lnr = sm_p.tile([128, 4, NMEM], F32, tag="lnr")
                pmtT = sm_p.tile([128, 4, NMEM], BF16, tag="pmtT")
                recip = sm_p.tile([128, 4, NMEM], F32, tag="recip")

                for g in range(NG):
                    m0 = g * GM
                    tr_sb = tr_p.tile([128, KT, GM, P2], F32R)
                    for kt in range(KT):
                        nc.gpsimd.dma_start(
                            out=tr_sb[:, kt],
                            in_=tf[s, m0:m0 + GM].rearrange(
                                "m (kt p) j -> p kt m j", p=128)[:, kt])
                    sq_sb = sq_p.tile([128, KT, GM, P2], BF16)
                    nc.vector.tensor_mul(sq_sb, tr_sb, tr_sb)
                    # ||tr_j||^2 with squares stationary -> j on partitions
                    for mi in range(GM):
                        m = m0 + mi
                        for ji, (j0, jn) in enumerate(JTS):
                            for kt in range(KT):
                                nc.tensor.matmul(
                                    r2[:jn, ji, m:m + 1],
                                    sq_sb[:, kt, mi, j0:j0 + jn],
                                    ones_bf,
                                    start=(g == 0 and mi == 0 and ji == 0
                                           and kt == 0),
                                    stop=(kt == KT - 1),
                                    skip_group_check=True)
                    gsl = slice(m0, m0 + GM)
                    nc.scalar.activation(lnr[:, :, gsl], r2[:, :, gsl], AF.Ln)
                    nc.scalar.activation(r_sb[:, :, gsl], lnr[:, :, gsl],
                                         AF.Exp, scale=-0.5)

                    for mi in range(GM):
                        m = m0 + mi
                        e_sb = e_p.tile([128, 4, P2], BF16)
                        for ji, (j0, jn) in enumerate(JTS):
                            s_ps = ps_mm1.tile([128, P2], F32)
                            for kt in range(KT):
                                nc.tensor.matmul(
                                    s_ps[:jn, :],
                                    tr_sb[:, kt, mi, j0:j0 + jn],
                                    te2[:, kt, :],
                                    start=(kt == 0), stop=(kt == KT - 1))
                            nc.scalar.activation(
                                e_sb[:jn, ji, :], s_ps[:jn, :], AF.Exp,
                                scale=r_sb[:jn, ji, m:m + 1])
                        # (numer, denom)^T accumulation over j-tiles
                        first = (m == 0)
                        for it, (i0, inn) in enumerate(JTS):
                            for ji, (j0, jn) in enumerate(JTS):
                                nc.tensor.matmul(
                                    pdt[:inn, it, m, :],
                                    e_sb[:jn, ji, i0:i0 + inn],
                                    w_sb[:jn, ji, :, m],
                                    start=(first and it == 0 and ji == 0),
                                    stop=(ji == 3),
                                    skip_group_check=True)

                # pmt^T = numer * 1/denom  (recip via exp(-ln))
                nc.scalar.activation(lnr, pdt[:, :, :, 1], AF.Ln)
                nc.scalar.activation(recip, lnr, AF.Exp, scale=-1.0)
                nc.vector.tensor_mul(pmtT, pdt[:, :, :, 0], recip)

                # ============ upsample + stats ============
                nc.sync.dma_start(
                    out=pd[s].rearrange("(it p) m -> p it m", p=128),
                    in_=pmtT)
                px = ep_p.tile([WF, HF, NMEM], BF16, tag="px")
                nc.sync.dma_start(
                    out=px, in_=pd[s, 0:P2].rearrange("(x y) m -> x y m",
                                                      y=HF))
                # up stage 1: per-m stationary, contract x -> (y=22, xL=88)
                for third in range(3):
                    u1ps = ps_s.tile([WF, 10, 128], F32, tag="scr")
                    for mi in range(10):
                        m = third * 10 + mi
                        nc.tensor.matmul(u1ps[:, mi, :WL],
                                         px[:, :, m], upt_bf,
                                         start=True, stop=True)
                    if third == 0:
                        o1 = ep_p.tile([WF, NMEM, WL], BF16, tag="o1")
                    nc.vector.tensor_copy(o1[:, third * 10:(third + 1) * 10, :],
                                          u1ps[:, :, :WL])
                # up stage 2: contract y: P = UP @ o1 -> (yL=88, (m,xL))
                acc1 = ep_p.tile([WL, HL], F32, tag="acc1")
                acc2 = ep_p.tile([WL, HL], F32, tag="acc2")
                for cch in range(6):
                    pps = ps_s.tile([WL, 512], F32, tag="scr")
                    nc.tensor.matmul(
                        pps[:, :440],
                        upt_bf,
                        o1[:, cch * 5:(cch + 1) * 5, :].rearrange(
                            "p m x -> p (m x)"),
                        start=True, stop=True)
                    pview = pps[:, :440].rearrange("p (m x) -> p x m", m=5)
                    red = epp_p.tile([WL, HL], F32, tag="red")
                    nc.vector.reduce_sum(red, pview, axis=mybir.AxisListType.X)
                    sqv = epp_p.tile([WL, 440], BF16, tag="sqv")
                    nc.scalar.activation(sqv, pps[:, :440], AF.Square)
                    red2 = epp_p.tile([WL, HL], F32, tag="red2")
                    nc.vector.reduce_sum(
                        red2, sqv.rearrange("p (m x) -> p x m", m=5),
                        axis=mybir.AxisListType.X)
                    if cch == 0:
                        nc.vector.tensor_copy(acc1, red)
                        nc.vector.tensor_copy(acc2, red2)
                    else:
                        nc.vector.tensor_add(acc1, acc1, red)
                        nc.vector.tensor_add(acc2, acc2, red2)
                # var = (acc2 - acc1^2/NMEM) / (NMEM-1); mean = acc1/NMEM
                m2 = ep_p.tile([WL, HL], F32, tag="m2")
                nc.vector.tensor_mul(m2, acc1, acc1)
                varb = ep_p.tile([WL, HL], F32, tag="varb")
                # varb = (acc2 - m2/NMEM) -> scalar_tensor_tensor:
                #   (m2 * -1/NMEM) + acc2
                nc.vector.scalar_tensor_tensor(
                    varb, m2, -1.0 / NMEM, acc2,
                    op0=mybir.AluOpType.mult, op1=mybir.AluOpType.add)
                # certainty = exp(a/(1+var) - a); 1+var = 1 + varb/29
                lnv = ep_p.tile([WL, HL], F32, tag="lnv")
                nc.scalar.activation(lnv, varb, AF.Ln, bias=1.0,
                                     scale=1.0 / (NMEM - 1))
                wrec = ep_p.tile([WL, HL], F32, tag="wrec")
                nc.scalar.activation(wrec, lnv, AF.Exp, scale=-1.0)
                cert = ep_p.tile([WL, HL], F32, tag="cert")
                nc.scalar.activation(cert, wrec, AF.Exp, bias=nalpha,
                                     scale=ALPHA)
                # our grid is (yL, xL) — transposed vs the (wl, hl) reference
                # layout, so ts/out go through small transposed DMAs
                ts_sb = ep_p.tile([WL, HL], F32, tag="ts_sb")
                with nc.allow_non_contiguous_dma(reason="31KB transposed io"):
                    nc.sync.dma_start(out=ts_sb,
                                      in_=ts[s].rearrange("x y -> y x"))
                o_sb = ep_p.tile([WL, HL], F32, tag="o_sb")
                # o = (acc1 * 1/NMEM) * cert
                nc.vector.scalar_tensor_tensor(
                    o_sb, acc1, 1.0 / NMEM, cert,
                    op0=mybir.AluOpType.mult, op1=mybir.AluOpType.mult)
                nc.vector.tensor_add(o_sb, o_sb, ts_sb)
                with nc.allow_non_contiguous_dma(reason="31KB transposed io"):
                    nc.sync.dma_start(out=out[s].rearrange("x y -> y x"),
                                      in_=o_sb)

    _split_excess_waits(nc)
    return nc


_NC_CACHE = None


def kernel(test_scores, train_labels, test_feat, train_feats, softmax_temp):
    global _NC_CACHE
    test_scores = np.asarray(test_scores, np.float32)
    train_labels = np.asarray(train_labels, np.float32)
    test_feat = np.asarray(test_feat, np.float32)
    train_feats = np.asarray(train_feats, np.float32)
    temp = np.asarray(softmax_temp, np.float32).reshape(1, 1)

    if _NC_CACHE is None:
        _NC_CACHE = build_program()
    nc = _NC_CACHE

    dnt_h = np.ascontiguousarray(_DN.T)            # (88, 22)
    upt_h = np.ascontiguousarray(_UP.T)            # (22, 88)

    in_maps = []
    for c in range(NCORES):
        sl = slice(c * SPC, (c + 1) * SPC)
        tf_c = np.ascontiguousarray(
            train_feats[:, sl].reshape(NMEM, SPC, C, P2).transpose(1, 0, 2, 3))
        te_c = np.ascontiguousarray(test_feat[0, sl].reshape(SPC, C, P2))
        lx_c = np.ascontiguousarray(
            train_labels[:, sl].transpose(1, 2, 0, 3))  # (SPC, 88, 30, 88)
        ts_c = np.ascontiguousarray(test_scores[0, sl])
        in_maps.append({
            "tf": tf_c, "te": te_c, "lx": lx_c, "ts": ts_c,
            "temp": temp, "dnt": dnt_h, "upt": upt_h,
        })

    res = run_bass_kernel_spmd(nc, in_maps, list(range(NCORES)))
    outs = [res.results[c]["out"] for c in range(NCORES)]
    return np.concatenate(outs, axis=0)[None].astype(np.float32)
